# revision 2
# baseline (speedup 1.0000x reference)
"""DerivativeNet (direction='x') on 8 Trainium2 NeuronCores — V3.

Contract: kernel(u, mask) takes FULL inputs
  u    [16, 2, 1024, 1024] f32
  mask [16, 1, 1024, 1024] f32
returns FULL output [16, 2, 1024, 1024] f32.

Sharding: data-parallel over batch — 2 samples per core, 8 cores.

Math per row along W (h=0.01, zero-padded u):
  d[k]   = up[k+1] - up[k],  k=0..W   (up = [0, u, 0])
  out[w] = pco[w]*d[w+1] + qco[w]*d[w]
  pco    = 50*er + 100*[cs==1]
  qco    = 50*er + 100*[cs[w]+cs[w-1] == 2T-1]     (== edge2, no mask mult)
  er     = [cs[w+1]-cs[w-2] == 3],  cs = cumsum(m), T = cs[W-1]

Engine split (HW-measured op costs):
  DVE : scans (f32->f16), box/er50/e1h/pco/yc/e2t/qco (f16), dsub/mul1/mul2
  ACT : u f32->f16 downconvert, PSUM->SBUF f32 drain, small edge copies,
        and the out-store DMA ring (HWDGE via scalar queue)
  PE  : final t1+t2 add via f16 identity matmuls into PSUM
  sync: all input loads
"""

import sys

if "/opt/trn_rl_repo" not in sys.path:
    sys.path.insert(0, "/opt/trn_rl_repo")

import numpy as np

_B, _C, _H, _W = 16, 2, 1024, 1024
_NCORES = 8
_BS = _B // _NCORES
_INV_H = 100.0
_INV_2H = 50.0

CFG = dict(
    S=2,                # h-segments per SBUF tile (tile = [128, S, 1024])
    ubufs=3,
    mbufs=2,
    pbufs=2,            # PSUM buffers (4 banks each at S=2)
    fadd_pe=True,       # final add on PE (else DVE f16 add + ACT upcast)
    udown_act=True,     # ACT downconverts u (else DVE dsub reads f32)
    store_eng="scalar",  # out-store DMA ring
    iters=1,
    dma_only=False,
)

_CACHE = {}


def _build_nc(cfg=None):
    import concourse.tile as tile
    from concourse import bacc, mybir
    from contextlib import ExitStack

    cfg = dict(CFG, **(cfg or {}))
    F32 = mybir.dt.float32
    F16 = mybir.dt.float16
    I32 = mybir.dt.int32
    Alu = mybir.AluOpType

    nc = bacc.Bacc("TRN2", target_bir_lowering=False, debug=False,
                   enable_asserts=False, num_devices=_NCORES)
    u_ap = nc.dram_tensor("u", [_BS, _C, _H, _W], F32,
                          kind="ExternalInput").ap()
    m_ap = nc.dram_tensor("mask", [_BS, _H, _W], F32,
                          kind="ExternalInput").ap()
    o_ap = nc.dram_tensor("out", [_BS, _C, _H, _W], F32,
                          kind="ExternalOutput").ap()

    P, S, W = 128, cfg["S"], _W
    R = P * S
    HT = _H // R
    fadd_pe = cfg["fadd_pe"]
    udown_act = cfg["udown_act"]
    e_store = {"scalar": nc.scalar, "sync": nc.sync}[cfg["store_eng"]]

    with tile.TileContext(nc) as tc:
        with ExitStack() as ctx:
            cpool = ctx.enter_context(tc.tile_pool(name="cn", bufs=1))
            mpool = ctx.enter_context(
                tc.tile_pool(name="mn", bufs=cfg["mbufs"]))
            upool = ctx.enter_context(
                tc.tile_pool(name="un", bufs=cfg["ubufs"]))
            if fadd_pe:
                ppool = ctx.enter_context(
                    tc.tile_pool(name="pp", bufs=cfg["pbufs"], space="PSUM"))

            # one-time: f16 identity for PE adds (iota row/col compare)
            if fadd_pe:
                ioj = cpool.tile([P, P], I32, tag="ioj")
                nc.gpsimd.iota(ioj[:], pattern=[[1, P]], base=0,
                               channel_multiplier=0)
                iop = cpool.tile([P, P], I32, tag="iop")
                nc.gpsimd.iota(iop[:], pattern=[[0, P]], base=0,
                               channel_multiplier=1)
                ident = cpool.tile([P, P], F16, tag="ident")
                nc.vector.tensor_tensor(ident[:], ioj[:], iop[:],
                                        Alu.is_equal)

            if cfg["iters"] > 1:
                loop_cm = tc.For_i(0, cfg["iters"], 1)
                ctx.enter_context(loop_cm)

            # software-pipelined emission: drains+stores for item i-1 are
            # emitted during item i, so ACT's FIFO never waits on PE/DVE.
            pending = []

            def flush_pending():
                while pending:
                    src_, odst_ = pending.pop(0)
                    o32 = upool.tile([P, S, W], F32, tag="o32")
                    nc.scalar.copy(o32[:], src_)
                    e_store.dma_start(odst_, o32[:])

            for b in range(_BS):
                for ht in range(HT):
                    r0 = ht * R
                    # ---- loads (sync ring) ----
                    m32 = mpool.tile([P, S, W], F32, tag="m32")
                    msrc = m_ap[b, r0:r0 + R, :].rearrange(
                        "(s p) w -> p s w", p=P)
                    nc.sync.dma_start(m32[:], msrc)
                    u32s = []
                    for c in range(_C):
                        u32 = upool.tile([P, S, W], F32, tag=f"u32_{c}")
                        usrc = u_ap[b, c, r0:r0 + R, :].rearrange(
                            "(s p) w -> p s w", p=P)
                        nc.sync.dma_start(u32[:], usrc)
                        u32s.append(u32)

                    if cfg["dma_only"]:
                        for c in range(_C):
                            odst = o_ap[b, c, r0:r0 + R, :].rearrange(
                                "(s p) w -> p s w", p=P)
                            e_store.dma_start(odst, u32s[c][:])
                        continue

                    # ---- ACT: u downconverts first (depend only on loads),
                    # then last item's PSUM drains + stores (deps long done)
                    ups = []
                    if udown_act:
                        for c in range(_C):
                            up = upool.tile([P, S, W + 2], F16, tag=f"up{c}")
                            nc.scalar.copy(up[:, :, 1:1 + W], u32s[c][:])
                            ups.append(up)
                    flush_pending()

                    # ---- mask pipeline (DVE only) ----
                    # csp[w+2] = cs[w]; csp[0:2] = 0; csp[2+W] = cs[W-1] dup
                    csp = mpool.tile([P, S, W + 4], F16, tag="csp")
                    nc.vector.memset(csp[:, :, 0:2], 0.0)
                    for s in range(S):
                        nc.vector.tensor_tensor_scan(
                            csp[:, s, 2:2 + W], m32[:, s, :], m32[:, s, :],
                            0.0, Alu.add, Alu.bypass)
                    nc.vector.tensor_copy(csp[:, :, 2 + W:3 + W],
                                          csp[:, :, 1 + W:2 + W])
                    # tot2 = 2T - 1 (f32, for the e2 compare)
                    tot2 = mpool.tile([P, S, 1], F32, tag="tot2")
                    nc.vector.tensor_scalar(tot2[:], csp[:, :, 1 + W:2 + W],
                                            2.0, -1.0, Alu.mult, Alu.add)

                    cs = csp[:, :, 2:2 + W]
                    # er50 = 50*[box == 3], box = csp[w+3] - csp[w]
                    er50 = mpool.tile([P, S, W], F16, tag="er50")
                    nc.vector.tensor_sub(er50[:], csp[:, :, 3:3 + W],
                                         csp[:, :, 0:W])
                    nc.vector.tensor_scalar(er50[:], er50[:], 2.5, _INV_2H,
                                            Alu.is_ge, Alu.mult)
                    # pco = er50 + 100*[cs == 1]
                    pco = mpool.tile([P, S, W], F16, tag="pco")
                    nc.vector.tensor_scalar(pco[:], cs, 1.0, _INV_H,
                                            Alu.is_equal, Alu.mult)
                    nc.vector.tensor_add(pco[:], pco[:], er50[:])
                    # qco = er50 + 100*[cs[w]+cs[w-1] == 2T-1]
                    qco = mpool.tile([P, S, W], F16, tag="qco")
                    nc.vector.tensor_add(qco[:], csp[:, :, 2:2 + W],
                                         csp[:, :, 1:1 + W])
                    for s in range(S):
                        nc.vector.tensor_scalar(qco[:, s, :], qco[:, s, :],
                                                tot2[:, s, :], _INV_H,
                                                Alu.is_equal, Alu.mult)
                    nc.vector.tensor_add(qco[:], qco[:], er50[:])

                    # ---- u pipeline (per channel) ----
                    for c in range(_C):
                        u32 = u32s[c]
                        d = upool.tile([P, S, W + 1], F16, tag="d")
                        if udown_act:
                            up = ups[c]
                            nc.vector.memset(up[:, :, 0:1], 0.0)
                            nc.vector.memset(up[:, :, W + 1:W + 2], 0.0)
                            nc.vector.tensor_sub(d[:], up[:, :, 1:W + 2],
                                                 up[:, :, 0:W + 1])
                        else:
                            # d[1..W-1] from u; d[0]=u[0], d[W]=-u[W-1]
                            nc.vector.tensor_sub(d[:, :, 1:W],
                                                 u32[:, :, 1:W],
                                                 u32[:, :, 0:W - 1])
                            nc.vector.tensor_copy(d[:, :, 0:1],
                                                  u32[:, :, 0:1])
                            nc.vector.tensor_scalar(
                                d[:, :, W:W + 1], u32[:, :, W - 1:W],
                                -1.0, 0.0, Alu.mult, Alu.add)
                        t1 = upool.tile([P, S, W], F16, tag="t1")
                        nc.vector.tensor_mul(t1[:], pco[:], d[:, :, 1:1 + W])
                        t2 = upool.tile([P, S, W], F16, tag="t2")
                        nc.vector.tensor_mul(t2[:], qco[:], d[:, :, 0:W])

                        odst = o_ap[b, c, r0:r0 + R, :].rearrange(
                            "(s p) w -> p s w", p=P)
                        if fadd_pe:
                            pt = ppool.tile([P, S, W], F32, tag="pt")
                            for s in range(S):
                                for j in range(0, W, 512):
                                    nc.tensor.matmul(
                                        pt[:, s, j:j + 512], ident[:],
                                        t1[:, s, j:j + 512],
                                        start=True, stop=False)
                                    nc.tensor.matmul(
                                        pt[:, s, j:j + 512], ident[:],
                                        t2[:, s, j:j + 512],
                                        start=False, stop=True)
                            pending.append((pt[:], odst))
                        else:
                            nc.vector.tensor_add(t1[:], t1[:], t2[:])
                            pending.append((t1[:], odst))
            flush_pending()
    nc.compile()
    return nc


def _act_copy(mybir):
    Act = mybir.ActivationFunctionType
    for name in ("Copy", "Identity", "Bypass"):
        if hasattr(Act, name):
            return getattr(Act, name)
    raise AttributeError("no copy-like activation function")


def _get_runner():
    if "runner" in _CACHE:
        return _CACHE["runner"]

    import jax
    from jax.sharding import Mesh, PartitionSpec
    from jax.experimental.shard_map import shard_map
    from concourse import bass2jax, mybir

    nc = _build_nc()
    bass2jax.install_neuronx_cc_hook()

    partition_name = (nc.partition_id_tensor.name
                      if nc.partition_id_tensor else None)
    in_names = []
    out_names = []
    out_avals = []
    zero_shapes = []
    for alloc in nc.m.functions[0].allocations:
        if not isinstance(alloc, mybir.MemoryLocationSet):
            continue
        name = alloc.memorylocations[0].name
        if alloc.kind == "ExternalInput":
            if name != partition_name:
                in_names.append(name)
        elif alloc.kind == "ExternalOutput":
            out_names.append(name)
            shape = tuple(alloc.tensor_shape)
            dtype = mybir.dt.np(alloc.dtype)
            out_avals.append(jax.core.ShapedArray(shape, dtype))
            zero_shapes.append((shape, dtype))
    n_params = len(in_names)
    all_names = in_names + out_names
    if partition_name is not None:
        all_names = all_names + [partition_name]

    def _body(*args):
        operands = list(args)
        if partition_name is not None:
            operands.append(bass2jax.partition_id_tensor())
        outs = bass2jax._bass_exec_p.bind(
            *operands,
            out_avals=tuple(out_avals),
            in_names=tuple(all_names),
            out_names=tuple(out_names),
            lowering_input_output_aliases=(),
            sim_require_finite=True,
            sim_require_nnan=True,
            nc=nc,
        )
        return tuple(outs)

    devices = jax.devices()[:_NCORES]
    mesh = Mesh(np.asarray(devices), ("core",))
    n_outs = len(out_names)
    sharded = jax.jit(
        shard_map(_body, mesh=mesh,
                  in_specs=(PartitionSpec("core"),) * (n_params + n_outs),
                  out_specs=(PartitionSpec("core"),) * n_outs,
                  check_rep=False),
        donate_argnums=tuple(range(n_params, n_params + n_outs)),
        keep_unused=True,
    )

    name_to_idx = {n: i for i, n in enumerate(in_names)}

    def run(u_full, mask_full):
        u_full = np.ascontiguousarray(u_full, dtype=np.float32)
        mask_full = np.ascontiguousarray(
            mask_full, dtype=np.float32).reshape(_B, _H, _W)
        args = [None] * n_params
        args[name_to_idx["u"]] = u_full
        args[name_to_idx["mask"]] = mask_full
        zeros = [np.zeros((_NCORES * s[0], *s[1:]), d)
                 for (s, d) in zero_shapes]
        out_arrs = sharded(*args, *zeros)
        out = np.asarray(out_arrs[out_names.index("out")])
        return out.reshape(_B, _C, _H, _W)

    _CACHE["runner"] = run
    return run


def kernel(u, mask):
    run = _get_runner()
    return run(u, mask)


if __name__ == "__main__":
    rng = np.random.default_rng(0)
    u = rng.standard_normal((_B, _C, _H, _W), dtype=np.float32)
    mask = (rng.random((_B, 1, _H, _W)) < 0.5).astype(np.float32)
    out = kernel(u=u, mask=mask)
    print("out", out.shape, out.dtype, float(np.abs(out).max()))
